# revision 18
# baseline (speedup 1.0000x reference)
"""BankModulatedConv Trainium2 kernel (v5).

Problem (per sample b of B=8, one NeuronCore per sample):
  w = softmax(bank_request[b])                        # (16,)
  kern = sum_f w[f] * bank_weight[f]                  # (o, i, kh, kw) = (256, 256, 3, 3)
  kern *= (1 + style[b, i])                           # input-channel modulation
  kern *= rsqrt(sum_{i,kh,kw} kern^2 + 1e-8)          # per-o L2 demodulation
  y[b] = conv2d(x[b], kern, stride 1, SAME)           # (256, 64, 64)

Mapping (data-parallel over batch; all math on device). Measured-rate
driven design (conv matmul ~217ns/512 cols; DVE bf16: tensor_scalar
~520ns, tensor_tensor ~760ns for 1152 elems; scalar_tensor_tensor and
custom DVE ops get no 2x mode -> avoided; Pool-engine ALU ops crash the
walrus backend -> avoided):
  - bank ships host-rearranged bf16 as 32 f-pair tiles
      [oc(2), ic(2), pair(8), i(128)] x [f_in_pair(2), o_local(128), khw(9)]
    fat 4608B rows; tile0 carries I_128 in 128 extra cols.
  - style distributes over the f-sum: sw[i, f] = w_f * (1 + style_i), so
    mixing produces the styled kernel directly.
  - mix(0,0): PE diag(sw_f) matmuls during the otherwise-idle DMA lead-in.
  - mix(0,1): PE again, but spliced between the two conv passes of oc0 and
    pumped through the single free aux PSUM bank in 3 sweeps (conv owns
    the other 7 banks); this converts a 16us PE wait into ~10us of work.
  - mix(1,0)/mix(1,1): bf16 multiply+add trees on DVE (16 tensor_scalar +
    15 tensor_tensor each; both ops have fast packed modes, unlike STT),
    paced behind their DMA windows. The last tree add lands in the conv
    lhsT tile.
  - conv: two 9-tap passes per o-chunk over spatial tiles s0..s6, then s7
    standalone in the aux bank. PSUM = 7 conv + 1 aux = 8.
  - demod: ScalarE Square -> DVE khw-group-reduce -> ones-matmul (aux
    bank) -> sqrt/recip -> K=1 matmul broadcast; scale applied in the
    ScalarE PSUM->bf16 y copy. x and y are bf16 on the wire.
"""
import sys

if "/opt/trn_rl_repo" not in sys.path:
    sys.path.insert(0, "/opt/trn_rl_repo")

import numpy as np
import concourse.bacc as bacc
import concourse.mybir as mybir
import concourse.tile as tile
from concourse.alu_op_type import AluOpType
from concourse.bass_utils import run_bass_kernel_spmd

dt = mybir.dt
AF = mybir.ActivationFunctionType

B, F, D, KK, H, W = 8, 16, 256, 3, 64, 64
HW = H * W            # 4096
KHW = KK * KK         # 9
IC = D // 128         # 2 i-chunks
OC = D // 128         # 2 o-chunks
NP = 8                # f-pairs per (oc, ic) block
FPP = 2               # f per pair
OCK = 128 * KHW       # 1152 free elems per (o_chunk, khw) group
PROW = FPP * OCK      # 2304 bf16 elems per bank pair-tile row
PW = W + 2            # padded width 66
PH_ = H + 2           # padded height 66
XN = PH_ * PW         # 4356
NS = 8                # spatial tiles (8 rows each)
SROWS = H // NS       # 8 rows per spatial tile
SN = SROWS * W        # 512 = conv matmul moving size
SL = ((0, 512), (512, 1024), (1024, OCK))   # psum sweep slice bounds

_COMPILED = None


def _build(num_devices=B):
    nc = bacc.Bacc("TRN2", target_bir_lowering=False, debug=False,
                   num_devices=num_devices)

    f32, bf16 = dt.float32, dt.bfloat16

    x_d = nc.dram_tensor("x", [D, XN], bf16, kind="ExternalInput").ap()
    # rows [oc, ic, pair, i]; first 128 rows carry 128 extra cols = I_128
    bank_d = nc.dram_tensor("bank", [OC * IC * NP * 128, PROW + 128], bf16,
                            kind="ExternalInput").ap()
    breq_d = nc.dram_tensor("breq", [1, F], f32, kind="ExternalInput").ap()
    sty_d = nc.dram_tensor("sty", [1, D], f32, kind="ExternalInput").ap()
    y_d = nc.dram_tensor("y", [D, HW], bf16, kind="ExternalOutput").ap()

    with tile.TileContext(nc) as tc:
        with (
            tc.tile_pool(name="setup", bufs=1) as setup,
            tc.tile_pool(name="xp", bufs=1) as xp,
            tc.tile_pool(name="bankp", bufs=1) as bankp,
            tc.tile_pool(name="kern", bufs=1) as kernp,
            tc.tile_pool(name="treep", bufs=2) as treep,
            tc.tile_pool(name="yout", bufs=3) as youtp,
            tc.tile_pool(name="auxps", bufs=1, space="PSUM") as auxps,
        ):
            # ---------------- DMA issue (sync queue) ----------------
            breq = setup.tile([1, F], f32)
            nc.sync.dma_start(breq[:], breq_d[:])
            styrow = setup.tile([1, D], f32)
            nc.sync.dma_start(styrow[:], sty_d[:])

            bts = {}

            def issue_bank(oc, ic, p):
                wide = (oc, ic, p) == (0, 0, 0)
                cols = PROW + 128 if wide else PROW
                tag = "bk00w" if wide else f"bk{oc}{ic}"
                b_t = bankp.tile([128, cols], bf16, tag=tag,
                                 bufs=1 if wide else 8 if (oc, ic) != (0, 0) else 7,
                                 name=f"bk{oc}{ic}p{p}")
                row0 = (((oc * IC + ic) * NP) + p) * 128
                nc.sync.dma_start(b_t[:], bank_d[row0:row0 + 128, 0:cols])
                bts[(oc, ic, p)] = b_t

            xpads = []
            xchunks = ((0, 17), (17, 34), (34, 51), (51, 66))

            def issue_x(ic, c):
                r0, r1 = xchunks[c]
                nc.sync.dma_start(xpads[ic][:, r0 * PW:r1 * PW],
                                  x_d[ic * 128:(ic + 1) * 128, r0 * PW:r1 * PW])

            for ic in range(IC):
                xpad = xp.tile([128, XN], bf16, tag=f"xpad{ic}", name=f"xpad{ic}")
                xpads.append(xpad)

            for p in range(NP):
                issue_bank(0, 0, p)
            issue_x(0, 0)
            issue_x(0, 1)
            issue_bank(0, 1, 0)
            issue_x(0, 2)
            issue_bank(0, 1, 1)
            issue_x(0, 3)
            for p in range(2, NP):
                issue_bank(0, 1, p)
            for p in range(NP):
                issue_bank(1, 0, p)
            for p in range(NP):
                issue_bank(1, 1, p)
            for c in range(4):
                issue_x(1, c)

            # ---------------- softmax + broadcast setup ----------------
            # Prefetch the Exp activation table before breq even lands.
            dum = setup.tile([1, 1], f32)
            nc.scalar.memzero(dum[:])
            dume = setup.tile([1, 1], f32)
            nc.scalar.activation(dume[:], dum[:], AF.Exp, bias=0.0, scale=1.0)

            # softmax without the max-shift: inputs are O(1) so exp is safe.
            ex = setup.tile([1, F], f32)
            nc.scalar.activation(ex[:], breq[:], AF.Exp, bias=0.0, scale=1.0)
            sm = setup.tile([1, 1], f32)
            nc.vector.reduce_sum(sm[:], ex[:], axis=mybir.AxisListType.X)
            rs = setup.tile([1, 1], f32)
            nc.vector.reciprocal(rs[:], sm[:])
            wrow = setup.tile([1, F], f32)
            nc.vector.tensor_scalar(out=wrow[:], in0=ex[:], scalar1=rs[:],
                                    scalar2=None, op0=AluOpType.mult)
            wrow_b = setup.tile([1, F], bf16)
            with nc.allow_low_precision(reason="broadcast weights only"):
                nc.vector.tensor_copy(wrow_b[:], wrow[:])

            onesrow_b = setup.tile([1, 128], bf16)
            nc.vector.memset(onesrow_b[:], 1.0)
            ones11_b = onesrow_b[0:1, 0:1]
            onescol_b = setup.tile([128, 1], bf16)
            nc.vector.memset(onescol_b[:], 1.0)
            ones11_f = setup.tile([1, 1], f32)
            nc.vector.memset(ones11_f[:], 1.0)

            # (1 + style) as a bf16 row for K=1 broadcast matmuls
            sty1 = setup.tile([1, D], f32)
            nc.scalar.activation(sty1[:], styrow[:], AF.Copy, bias=1.0, scale=1.0)
            sty1b = setup.tile([1, D], bf16)
            with nc.allow_low_precision(reason="style factors, bf16 like bank"):
                nc.vector.tensor_copy(sty1b[:], sty1[:])

            # aux psum: w broadcast (cols 0:16) + style columns (cols 16:18)
            aux0 = auxps.tile([128, 512], f32, tag="aux", name="aux0")
            nc.tensor.matmul(aux0[:, 0:F], onesrow_b[:], wrow_b[:],
                             start=True, stop=True)
            for ic in range(IC):
                nc.tensor.matmul(aux0[:, F + ic:F + ic + 1],
                                 sty1b[0:1, ic * 128:(ic + 1) * 128],
                                 ones11_b, start=True, stop=True)
            wbc = setup.tile([128, F], f32)
            nc.vector.tensor_copy(wbc[:], aux0[:, 0:F])
            # style-folded weights: sw[i, f] = w_f * (1 + style_i), per ic
            sws = []
            for ic in range(IC):
                sw = setup.tile([128, F], f32, tag=f"sw{ic}", name=f"sw{ic}")
                nc.vector.tensor_scalar(out=sw[:], in0=wbc[:],
                                        scalar1=aux0[:, F + ic:F + ic + 1],
                                        scalar2=None, op0=AluOpType.mult)
                sws.append(sw)

            ident = bts[(0, 0, 0)][:, PROW:PROW + 128]

            def build_diags(ic):
                dgs = []
                with nc.allow_low_precision(reason="bf16 diag weights"):
                    for f in range(F):
                        dg = setup.tile([128, 128], bf16, tag="diag", bufs=2 * F,
                                        name=f"dg{ic}_{f}")
                        nc.vector.tensor_scalar(out=dg[:], in0=ident[:],
                                                scalar1=sws[ic][:, f:f + 1],
                                                scalar2=None, op0=AluOpType.mult)
                        dgs.append(dg)
                return dgs

            diags0 = build_diags(0)

            km = {}
            redks = {}
            rsums = {}
            ncols = {}

            # ---------------- mix block (0,0) on the PE ----------------
            kt00 = kernp.tile([128, OCK], bf16, tag="kt00", name="kt00")
            with tc.tile_pool(name="mixps", bufs=1, space="PSUM") as mixps:
                ps0 = mixps.tile([128, 512], f32, tag="m0", name="m0")
                ps1 = mixps.tile([128, 512], f32, tag="m1", name="m1")
                ps2 = mixps.tile([128, OCK - 1024], f32, tag="m2", name="m2")
                pss = (ps0, ps1, ps2)
                for p in range(NP):
                    b_t = bts[(0, 0, p)]
                    for fl in range(FPP):
                        f = p * FPP + fl
                        fo = fl * OCK
                        for (lo, hi), ps in zip(SL, pss):
                            nc.tensor.matmul(ps[:], diags0[f][:],
                                             b_t[:, fo + lo:fo + hi],
                                             start=(f == 0), stop=(f == F - 1))
                with nc.allow_low_precision(reason="kernel storage bf16"):
                    for (lo, hi), ps in zip(SL, pss):
                        nc.scalar.activation(kt00[:, lo:hi], ps[:], AF.Copy,
                                             bias=0.0, scale=1.0)
            km[(0, 0)] = kt00

            # ---------------- demod: ScalarE square, DVE group-reduce ----
            def demod_sq(oc, ic):
                kt = km[(oc, ic)]
                scr = kernp.tile([128, OCK], bf16, tag="scr", name=f"scr{oc}{ic}")
                with nc.allow_low_precision(reason="demod stats in bf16"):
                    nc.scalar.activation(scr[:], kt[:], AF.Square,
                                         bias=0.0, scale=1.0)
                return scr

            def demod_red(oc, ic, scr):
                redk = kernp.tile([128, 128], bf16, tag="redk", bufs=2,
                                  name=f"redk{oc}{ic}")
                with nc.allow_low_precision(reason="demod stats in bf16"):
                    nc.vector.tensor_reduce(
                        redk[:], scr[:, :].rearrange("p (o r) -> p o r", r=KHW),
                        axis=mybir.AxisListType.X, op=AluOpType.add)
                redks[(oc, ic)] = redk

            def rsum_of(oc):
                rsum = kernp.tile([128, 128], bf16, tag="rsum", bufs=2,
                                  name=f"rsum{oc}")
                with nc.allow_low_precision(reason="demod stats in bf16"):
                    nc.vector.tensor_tensor(out=rsum[:], in0=redks[(oc, 0)][:],
                                            in1=redks[(oc, 1)][:],
                                            op=AluOpType.add)
                rsums[oc] = rsum

            scr00 = demod_sq(0, 0)
            demod_red(0, 0, scr00)
            diags1 = build_diags(1)

            # ---------------- bf16 mix trees on DVE ----------------
            # kernel = sum_f sw_f*B_f: 16 tensor_scalar (fast packed mode)
            # + 8 pair-adds + 7 running-sum adds, all bf16 on DVE. The 7th
            # running sum writes the kt tile.
            def tree(oc, ic, pair_hook=None):
                kt = kernp.tile([128, OCK], bf16, tag=f"kt{oc}{ic}",
                                name=f"kt{oc}{ic}")
                acs = kernp.tile([128, OCK], bf16, tag="acs", name=f"acs{oc}{ic}")
                sw = sws[ic]
                acc = None
                with nc.allow_low_precision(reason="bf16 mix tree"):
                    for p in range(NP):
                        b_t = bts[(oc, ic, p)]
                        ta = treep.tile([128, OCK], bf16, tag="t0", name=f"ta{p}")
                        nc.vector.tensor_scalar(
                            out=ta[:], in0=b_t[:, 0:OCK],
                            scalar1=sw[:, 2 * p:2 * p + 1],
                            scalar2=None, op0=AluOpType.mult)
                        tb = treep.tile([128, OCK], bf16, tag="t1", name=f"tb{p}")
                        nc.vector.tensor_scalar(
                            out=tb[:], in0=b_t[:, OCK:2 * OCK],
                            scalar1=sw[:, 2 * p + 1:2 * p + 2],
                            scalar2=None, op0=AluOpType.mult)
                        if p == 0:
                            acc = acs
                            nc.vector.tensor_tensor(out=acs[:], in0=ta[:],
                                                    in1=tb[:], op=AluOpType.add)
                        else:
                            u = treep.tile([128, OCK], bf16, tag="u", name=f"u{p}")
                            nc.vector.tensor_tensor(out=u[:], in0=ta[:],
                                                    in1=tb[:], op=AluOpType.add)
                            nxt = acs if acc is kt else kt
                            nc.vector.tensor_tensor(out=nxt[:], in0=acc[:],
                                                    in1=u[:], op=AluOpType.add)
                            acc = nxt
                        if pair_hook is not None:
                            pair_hook(p)
                assert acc is kt
                km[(oc, ic)] = kt

            # ---------------- conv ----------------
            xvs = [xpads[ic][:, :].rearrange("p (r c) -> p r c", c=PW)
                   for ic in range(IC)]

            def taps(cps, oc, ic, s, first, last):
                xv = xvs[ic]
                kv = km[(oc, ic)][:, :].rearrange("p (o r) -> p o r", r=KHW)
                r0 = s * SROWS
                for kh in range(KK):
                    for kw in range(KK):
                        nc.tensor.matmul(
                            cps[:], kv[:, :, kh * KK + kw],
                            xv[:, r0 + kh:r0 + kh + SROWS, kw:kw + W],
                            start=(first and kh == 0 and kw == 0),
                            stop=(last and kh == KK - 1 and kw == KK - 1))

            def norm_mm(oc):
                npsum = auxps.tile([128, 512], f32, tag="aux", name=f"np{oc}")
                nc.tensor.matmul(npsum[0:1, 0:128], onescol_b[:], rsums[oc][:],
                                 start=True, stop=True)
                nrow = setup.tile([1, 128], f32, tag=f"nrow{oc}", name=f"nrow{oc}")
                nc.scalar.activation(nrow[:], npsum[0:1, 0:128], AF.Copy,
                                     bias=1e-8, scale=1.0)
                nsq = setup.tile([1, 128], f32, tag=f"nsq{oc}", name=f"nsq{oc}")
                nc.scalar.activation(nsq[:], nrow[:], AF.Sqrt,
                                     bias=0.0, scale=1.0)
                nrec = setup.tile([1, 128], f32, tag=f"nrec{oc}", name=f"nrec{oc}")
                nc.vector.reciprocal(nrec[:], nsq[:])
                return nrec

            def ntr_mm(oc, nrec):
                ntr = auxps.tile([128, 512], f32, tag="aux", name=f"ntr{oc}")
                nc.tensor.matmul(ntr[:, 0:1], nrec[:], ones11_f[:],
                                 start=True, stop=True)
                ncol = setup.tile([128, 1], f32, tag=f"ncol{oc}", name=f"ncol{oc}")
                nc.scalar.activation(ncol[:], ntr[:, 0:1], AF.Copy,
                                     bias=0.0, scale=1.0)
                ncols[oc] = ncol

            def yout(oc, s, cps):
                yt = youtp.tile([128, SN], bf16, tag="y", name=f"y{oc}{s}")
                with nc.allow_low_precision(reason="y storage bf16"):
                    nc.scalar.activation(yt[:], cps[:], AF.Copy,
                                         bias=0.0, scale=ncols[oc][:])
                nc.scalar.dma_start(
                    y_d[oc * 128:(oc + 1) * 128, s * SN:(s + 1) * SN], yt[:])

            with tc.tile_pool(name="convps", bufs=7, space="PSUM") as convps:

                def pass1(oc):
                    cpss = []
                    for s in range(7):
                        cps = convps.tile([128, SN], f32, tag="c", name=f"c{oc}{s}")
                        taps(cps, oc, 0, s, first=True, last=False)
                        cpss.append(cps)
                    return cpss

                def pass2(oc, cpss, norm_s, ntr_s):
                    nrec = None
                    for s in range(7):
                        taps(cpss[s], oc, 1, s, first=False, last=True)
                        if s == norm_s:
                            nrec = norm_mm(oc)
                        elif s == ntr_s:
                            ntr_mm(oc, nrec)
                            for t in range(s + 1):
                                yout(oc, t, cpss[t])
                        elif s > ntr_s:
                            yout(oc, s, cpss[s])

                def conv_s7(oc):
                    cps7 = auxps.tile([128, SN], f32, tag="aux", name=f"c{oc}7")
                    taps(cps7, oc, 0, 7, first=True, last=False)
                    taps(cps7, oc, 1, 7, first=False, last=True)
                    yout(oc, 7, cps7)

                # -------- oc0 --------
                cpss0 = pass1(0)

                # mix(0,1) on PE through the aux psum bank, 3 sweeps
                kt01 = kernp.tile([128, OCK], bf16, tag="kt01", name="kt01")
                for (lo, hi) in SL:
                    swp = auxps.tile([128, 512], f32, tag="aux",
                                     name=f"sw{lo}")
                    cols = hi - lo
                    for p in range(NP):
                        b_t = bts[(0, 1, p)]
                        for fl in range(FPP):
                            f = p * FPP + fl
                            fo = fl * OCK
                            nc.tensor.matmul(swp[:, 0:cols], diags1[f][:],
                                             b_t[:, fo + lo:fo + hi],
                                             start=(f == 0), stop=(f == F - 1))
                    with nc.allow_low_precision(reason="kernel storage bf16"):
                        nc.scalar.activation(kt01[:, lo:hi], swp[:, 0:cols],
                                             AF.Copy, bias=0.0, scale=1.0)
                km[(0, 1)] = kt01
                scr01 = demod_sq(0, 1)

                # DVE: tree(1,0) with the oc0-norm work spliced after pair 5
                def hook10(p):
                    if p == 5:
                        demod_red(0, 1, scr01)
                        rsum_of(0)

                tree(1, 0, pair_hook=hook10)

                pass2(0, cpss0, norm_s=2, ntr_s=5)
                scr10 = demod_sq(1, 0)
                conv_s7(0)

                # DVE: tree(1,1) fully, then the pending reduces
                tree(1, 1)
                scr11 = demod_sq(1, 1)
                demod_red(1, 0, scr10)
                demod_red(1, 1, scr11)
                rsum_of(1)

                # -------- oc1 --------
                cpss1 = pass1(1)
                pass2(1, cpss1, norm_s=2, ntr_s=3)
                conv_s7(1)

    nc.compile()
    return nc


def _get_compiled():
    global _COMPILED
    if _COMPILED is None:
        _COMPILED = _build()
    return _COMPILED


def _make_in_maps(x, bank_request, style, bank_weight):
    bf16_np = mybir.dt.np(mybir.dt.bfloat16)
    # bank (f, o, i, kh, kw) -> rows [oc, ic, pair, i] x cols [fl, o_local, khw]
    A = bank_weight.astype(np.float32).reshape(NP, FPP, OC, 128, IC, 128, KHW)
    #                      dims: (pair, fl, oc, o_local, ic, i, khw)
    core = A.transpose(2, 4, 0, 5, 1, 3, 6).reshape(OC * IC * NP * 128, PROW)
    bankT = np.zeros((OC * IC * NP * 128, PROW + 128), dtype=np.float32)
    bankT[:, 0:PROW] = core
    bankT[0:128, PROW:PROW + 128] = np.eye(128, dtype=np.float32)
    bankT = np.ascontiguousarray(bankT).astype(bf16_np)

    xpad = np.zeros((B, D, PH_, PW), dtype=np.float32)
    xpad[:, :, 1:1 + H, 1:1 + W] = x.astype(np.float32).reshape(B, D, H, W)
    xpad = xpad.reshape(B, D, XN).astype(bf16_np)

    maps = []
    for b in range(B):
        maps.append({
            "x": np.ascontiguousarray(xpad[b]),
            "bank": bankT,
            "breq": np.ascontiguousarray(
                bank_request[b].astype(np.float32).reshape(1, F)),
            "sty": np.ascontiguousarray(style[b].astype(np.float32).reshape(1, D)),
        })
    return maps


def run(inputs, trace=False, **trace_kwargs):
    nc = _get_compiled()
    in_maps = _make_in_maps(inputs["x"], inputs["bank_request"],
                            inputs["style"], inputs["bank_weight"])
    # The first execution of a freshly compiled NEFF occasionally dies with
    # NRT_EXEC_UNIT_UNRECOVERABLE on this runtime; a plain retry succeeds.
    last_exc = None
    for _ in range(3):
        try:
            res = run_bass_kernel_spmd(nc, in_maps, core_ids=list(range(B)),
                                       trace=trace, **trace_kwargs)
            y = np.stack(
                [res.results[b]["y"].astype(np.float32).reshape(D, H, W)
                 for b in range(B)], axis=0)
            return y, res
        except Exception as e:  # noqa: BLE001
            last_exc = e
    raise last_exc


def kernel(x, bank_request, style, bank_weight):
    y, _ = run({"x": np.asarray(x), "bank_request": np.asarray(bank_request),
                "style": np.asarray(style), "bank_weight": np.asarray(bank_weight)})
    return y
